# revision 13
# baseline (speedup 1.0000x reference)
"""CTRNN scan kernel for Trainium2 (Bass/Tile), 8-core data-parallel.

Recurrence: h_t = relu((1-a)*h_{t-1} + a*(x_t @ W_in^T + b_in + h_{t-1} @ W_hh^T + b_hh))

Device-side formulation (per core, 32 batch rows):
  state kept transposed: g[k, b] = h[b, k]  (H=64 partitions)
  augmented rhs per step: [g_{t-1} (64p); x_t^T (32p); ones (1p)]  -> 97 partitions
  one matmul per step:  psum[j, b] = A_aug^T @ rhs   (A_aug is [97, 64])
  one DVE op per step:  g_t = max(psum, 0) written straight into the next
  rhs column block of a chunked SBUF "panel" [128, TC*32].

Host side does layout-only transforms (transpose x per core; transpose the
[H, T*BL] device output back to [B, T, H]).
"""

import numpy as np

DT, TAU = 16.67, 40.0
ALPHA = DT / TAU
OMA = 1.0 - ALPHA

B, T, D, H = 256, 2048, 32, 64
NCORES = 8
BL = B // NCORES          # 32 batch rows per core
K = H + D + 1             # 97 augmented contraction dim
TC = 256                  # timesteps per SBUF panel chunk
NCHUNK = T // TC

_CACHE = {}
_LAST_EXEC_NS = None
_LAST_RESULTS = None


def _build_nc(reps=1):
    import concourse.bacc as bacc
    import concourse.mybir as mybir
    from concourse.tile import TileContext

    f32 = mybir.dt.float32
    nc = bacc.Bacc("TRN2", target_bir_lowering=False)

    xq = nc.dram_tensor("xq", [D + 1, T * BL], f32, kind="ExternalInput")
    aug = nc.dram_tensor("aug", [K, H], f32, kind="ExternalInput")
    outT = nc.dram_tensor("outT", [H, T * BL], f32, kind="ExternalOutput")

    with TileContext(nc) as tc:
        with (
            tc.tile_pool(name="const", bufs=1) as cpool,
            tc.tile_pool(name="panels", bufs=3) as ppool,
            tc.tile_pool(name="stub", bufs=1) as spool,
            tc.tile_pool(name="ps", bufs=4, space="PSUM") as qpool,
        ):
            a_tile = cpool.tile([K, H], f32)
            nc.sync.dma_start(out=a_tile[:], in_=aug[:])

            for rep in range(reps):
                panels = [
                    ppool.tile(
                        [128, TC * BL], f32, tag="panel", name=f"panel{rep}_{_k}"
                    )
                    for _k in range(NCHUNK)
                ]
                stub = spool.tile([128, BL], f32, tag="stub", name=f"stub{rep}")

                # x (+ ones row baked into xq row D) into panel rows 64:97
                for k in range(NCHUNK):
                    nc.sync.dma_start(
                        out=panels[k][H:K, :],
                        in_=xq[:, k * TC * BL : (k + 1) * TC * BL],
                    )

                for t in range(T):
                    k, i = divmod(t, TC)
                    ps = qpool.tile([H, BL], f32, tag="ps", name=f"ps{rep}_{t}")
                    if t == 0:
                        # h0 = 0: step 0 needs only the x+ones rows
                        nc.tensor.matmul(
                            ps[:],
                            lhsT=a_tile[H:K, :],
                            rhs=panels[0][H:K, 0:BL],
                            start=True,
                            stop=True,
                        )
                    else:
                        rhs = panels[k][0:K, i * BL : (i + 1) * BL]
                        nc.tensor.matmul(
                            ps[:], lhsT=a_tile[:], rhs=rhs, start=True, stop=True
                        )
                    if i + 1 < TC:
                        dst = panels[k][0:H, (i + 1) * BL : (i + 2) * BL]
                    elif k + 1 < NCHUNK:
                        dst = panels[k + 1][0:H, 0:BL]
                    else:
                        dst = stub[0:H, 0:BL]
                    nc.vector.tensor_scalar_max(dst, ps[:], 0.0)

                # outputs: panel k block i holds g_{k*TC+i-1}
                nc.sync.dma_start(
                    out=outT[:, 0 : (TC - 1) * BL], in_=panels[0][0:H, BL:]
                )
                for k in range(1, NCHUNK):
                    base = (k * TC - 1) * BL
                    nc.sync.dma_start(
                        out=outT[:, base : base + TC * BL], in_=panels[k][0:H, :]
                    )
                nc.sync.dma_start(out=outT[:, (T - 1) * BL :], in_=stub[0:H, :])

    nc.compile()
    return nc


def _get_nc():
    if "nc" not in _CACHE:
        _CACHE["nc"] = _build_nc()
    return _CACHE["nc"]


def _make_in_maps(x, W_in, b_in, W_hh, b_hh):
    # A_aug[k, j]: rows 0:64 = (1-a)I + a*W_hh^T ; rows 64:96 = a*W_in^T ;
    # row 96 = a*(b_in + b_hh).  psum = A_aug^T @ [g; xT; 1] gives the full
    # pre-activation in transposed (j, b) layout.
    aug = np.empty((K, H), dtype=np.float32)
    aug[0:H] = OMA * np.eye(H, dtype=np.float32) + ALPHA * W_hh.T
    aug[H : H + D] = ALPHA * W_in.T
    aug[H + D] = ALPHA * (b_in + b_hh)

    in_maps = []
    for c in range(NCORES):
        xc = x[c * BL : (c + 1) * BL]                      # [BL, T, D]
        xq = np.empty((D + 1, T, BL), dtype=np.float32)
        xq[:D] = xc.transpose(2, 1, 0)                     # [D, T, BL]
        xq[D] = 1.0                                        # ones row for bias
        in_maps.append({"xq": xq.reshape(D + 1, T * BL), "aug": aug})
    return in_maps


def kernel(x, seq_lengths, W_in, b_in, W_hh, b_hh):
    from concourse.bass_utils import run_bass_kernel_spmd

    x = np.asarray(x, dtype=np.float32)
    W_in = np.asarray(W_in, dtype=np.float32)
    b_in = np.asarray(b_in, dtype=np.float32)
    W_hh = np.asarray(W_hh, dtype=np.float32)
    b_hh = np.asarray(b_hh, dtype=np.float32)

    in_maps = _make_in_maps(x, W_in, b_in, W_hh, b_hh)

    nc = _get_nc()
    res = run_bass_kernel_spmd(nc, in_maps, core_ids=list(range(NCORES)))
    global _LAST_EXEC_NS, _LAST_RESULTS
    _LAST_EXEC_NS = res.exec_time_ns
    _LAST_RESULTS = res

    outputs = np.empty((B, T, H), dtype=np.float32)
    for c in range(NCORES):
        oT = res.results[c]["outT"].reshape(H, T, BL)      # [H, T, BL]
        outputs[c * BL : (c + 1) * BL] = oT.transpose(2, 1, 0)
    h_last = outputs[:, -1, :].copy()
    return outputs, h_last
